# revision 15
# baseline (speedup 1.0000x reference)
"""Multi-head attention (N=2, K=2048, DIN=1024, H=16, DOUT=64) on 8 TRN2 NeuronCores.

Sharding: data-parallel over batch N (cores 0-3 -> n=0, cores 4-7 -> n=1),
tensor-parallel over heads (4 heads per core). Each core computes its 4 heads'
attention plus the partial output projection over its head-feature slice of Wp;
the host sums the 4 partials per batch element and adds the output bias.

Per-core kernel structure (all matmuls bf16, fp32 PSUM accumulation):
  - host pre-transposes/casts activations to bf16 [DIN, SEQ] so projection
    matmuls contract over DIN on partitions with natural contiguous DMA.
  - q/k projected head-pair-wise to [dout(2 heads on partitions), seq];
    v projected to the natural [seq, 4*64] layout.
  - scores computed transposed (S.T = k_h @ q_h.T: seq_k on partitions), so
    softmax probabilities are already in the layout the PV matmul needs.
  - no max-subtraction: scores are bounded (~|S/8| < 10), exp is safe.
  - softmax numerator is produced by TWO engine paths, alternating per
    seq_k tile (kt) so ScalarE, VectorE and GpSimd all share the exp load
    instead of ScalarE serializing the whole attention phase:
      * ACT path (9 of 16 kts): ScalarE spline exp reads S from PSUM
        (scale=1/8 plus a mean-matching bias, see below); the 0/1 mask
        multiply runs on GpSimd for most tiles, VectorE for the rest.
      * STT path (7 of 16 kts): one fused VectorE scalar_tensor_tensor
        computes a Schraudolph fast-exp WITH the mask applied:
        int16(S*(2^7*log2e/8) + B[k,q]) bitcast as bf16, where the host
        baked B = 16256 (valid -> exp) / -57344 (masked -> the fp32->int16
        convert saturates to -32768 = 0x8000 = bf16 -0.0). The linear-
        mantissa approximation is exact in the exponent and overestimates
        by mean +4.07% in the mantissa; the ACT path's exp bias
        ln(1.0407) matches that mean so the two representations can mix
        within one softmax sum (verified: rel err 1.28e-2 vs 2e-2 budget).
    The host interleaves the two mask encodings row-block-wise in ONE
    [SEQ, SEQ] bf16 tensor, so each kt DMAs only the form it needs.
  - the two heads' PV matmuls col-pack into one PSUM bank; softmax
    denominators accumulate via col-packed M=1 ones-matmuls in a separate
    bank, doubling as PE filler that keeps the HAM clock gate at 8/8.
  - a dependency-free warmup burst of full-array matmuls at kernel start
    brings the clock gate to 8/8 before the first DMA-fed matmuls arrive.
  - PSUM accumulators are DVE-zeroed and all accumulating matmuls use
    start=False.
  - normalization: the 4 denominator rows (psum partitions 0/32/64/96) are
    pulled with ONE [97,512] copy + ONE fast-reciprocal, broadcast via K=1
    outer-product matmuls into a col-packed [128,512] tile, and applied
    with one full-height multiply per qi.
  - epilogues are deferred and spread through the NEXT block's matmul
    stream so the PE never idles long enough to re-throttle.
"""

import numpy as np
import ml_dtypes

import concourse.bass as bass
import concourse.mybir as mybir
from concourse import bacc
from concourse.tile import TileContext

P = 128
SEQ = 2048
DIN = 1024
DOUT = 64
H = 16
N = 2
HPC = 4  # heads per core
NPAIR = 2  # head pairs per core
KSUB = DIN // P  # 8 contraction subtiles for projections
NKT = SEQ // P  # 16 seq_k tiles of 128
BF = mybir.dt.bfloat16
F32 = mybir.dt.float32
I16 = mybir.dt.int16
BF_NP = ml_dtypes.bfloat16

# Schraudolph fast-exp constants (see module docstring)
A_STT = 0.125 * float(np.log2(np.e)) * 128.0  # 23.0831...
B_VALID = 16256.0  # 127 * 2^7, bf16-exact
B_MASKED = -57344.0  # saturates the int16 convert -> 0x8000 -> bf16 -0.0
ACT_BIAS = 0.039883  # ln(1.04068): mean ratio of fast-exp vs exp
# kts produced by the fused VectorE fast-exp path (rest go via ScalarE).
# kt 14 is STT and kt 15's masks run on DVE so the block tail's masked
# probabilities arrive with short latency (no PE stall at the boundary).
STT_KTS = frozenset((3, 5, 7, 9, 11, 14))
# ACT-path tiles whose 0/1 mask multiply runs on GpSimd (engine balance;
# kt 15 stays on DVE for tail latency)
GP_MASK = frozenset(
    [(kt, 0) for kt in (0, 2, 4, 6, 8, 10, 12, 13)] + [(0, 1), (2, 1)]
)

_NC_CACHE = None


def build_bass():
    nc = bacc.Bacc()

    xq_d = nc.declare_dram_parameter("xqT", [DIN, SEQ], BF, isOutput=False)
    xk_d = nc.declare_dram_parameter("xkT", [DIN, SEQ], BF, isOutput=False)
    xv_d = nc.declare_dram_parameter("xvT", [DIN, SEQ], BF, isOutput=False)
    mk_d = nc.declare_dram_parameter("maskT", [SEQ, SEQ], BF, isOutput=False)
    wq_d = nc.declare_dram_parameter("wq", [DIN, HPC * DOUT], BF, isOutput=False)
    wk_d = nc.declare_dram_parameter("wk", [DIN, HPC * DOUT], BF, isOutput=False)
    wv_d = nc.declare_dram_parameter("wv", [DIN, HPC * DOUT], BF, isOutput=False)
    wp_d = nc.declare_dram_parameter("wp", [HPC * DOUT, DIN], BF, isOutput=False)
    bqp_d = nc.declare_dram_parameter("bqp", [P, NPAIR], F32, isOutput=False)
    bkp_d = nc.declare_dram_parameter("bkp", [P, NPAIR], F32, isOutput=False)
    bvr_d = nc.declare_dram_parameter("bvr", [P, HPC * DOUT], F32, isOutput=False)
    out_d = nc.declare_dram_parameter("out", [SEQ, DIN], F32, isOutput=True)

    ADD = mybir.AluOpType.add
    MUL = mybir.AluOpType.mult
    EXP = mybir.ActivationFunctionType.Exp

    with TileContext(nc) as tc:
        with (
            tc.tile_pool(name="const", bufs=1) as const,
            tc.tile_pool(name="xin", bufs=1) as xin,
            tc.tile_pool(name="proj", bufs=1) as proj,
            tc.tile_pool(name="maskp", bufs=4) as maskp,
            tc.tile_pool(name="ptp", bufs=2) as ptp,
            tc.tile_pool(name="epi", bufs=2) as epi,
            tc.tile_pool(name="ps_s", bufs=2, space="PSUM") as ps_s,
            tc.tile_pool(name="ps_pv", bufs=2, space="PSUM") as ps_pv,
            tc.tile_pool(name="ps_dn", bufs=2, space="PSUM") as ps_dn,
        ):
            # ---- constants -------------------------------------------------
            wq_sb = const.tile([P, KSUB, HPC * DOUT], BF)
            nc.sync.dma_start(wq_sb[:], wq_d.rearrange("(o p) m -> p o m", p=P))
            wk_sb = const.tile([P, KSUB, HPC * DOUT], BF)
            nc.sync.dma_start(wk_sb[:], wk_d.rearrange("(o p) m -> p o m", p=P))
            wv_sb = const.tile([P, KSUB, HPC * DOUT], BF)
            nc.sync.dma_start(wv_sb[:], wv_d.rearrange("(o p) m -> p o m", p=P))
            wp_sb = const.tile([P, NPAIR, DIN], BF)
            nc.sync.dma_start(wp_sb[:], wp_d.rearrange("(o p) n -> p o n", p=P))
            bqp_sb = const.tile([P, NPAIR], F32)
            nc.sync.dma_start(bqp_sb[:], bqp_d[:])
            bkp_sb = const.tile([P, NPAIR], F32)
            nc.sync.dma_start(bkp_sb[:], bkp_d[:])
            bvr_sb = const.tile([P, HPC * DOUT], F32)
            nc.sync.dma_start(bvr_sb[:], bvr_d[:])
            ones_sb = const.tile([P, 1], BF)
            nc.vector.memset(ones_sb[:], 1.0)
            # per-partition bias AP for the ACT-path exp (mean-matching)
            actb_sb = const.tile([P, 1], F32)
            nc.vector.memset(actb_sb[:], ACT_BIAS)
            # full-height fp32 ones: rows at 0/32/64/96 serve as the K=1
            # outer-product lhsT for the denominator broadcast
            ones64_sb = const.tile([P, DOUT], F32)
            nc.vector.memset(ones64_sb[:], 1.0)

            # PE warmup burst: dependency-free full-array matmuls at kernel
            # start so the HAM clock gate reaches 8/8 before the first real
            # (DMA-fed) matmuls arrive; the result is never read.
            warm_rhs = const.tile([P, 512], BF)
            nc.vector.memset(warm_rhs[:], 1.0)

            def warm_fill(n):
                warm_ps = ps_s.tile([P, 1024], F32, tag="s", name="warm_ps")
                for i in range(n):
                    nc.tensor.matmul(
                        warm_ps[:, 0:256],
                        warm_rhs[:, 0:P],
                        warm_rhs[:, 0:256],
                        start=True,
                        stop=True,
                        skip_group_check=True,
                    )

            warm_fill(80)

            # ---- resident transposed activations ---------------------------
            # chunked per DIN-subtile so the first projection matmuls can
            # start as soon as the first 512KB lands
            xq_sb = xin.tile([P, KSUB, SEQ], BF)
            xk_sb = xin.tile([P, KSUB, SEQ], BF)
            xv_sb = xin.tile([P, KSUB, SEQ], BF)
            for x_sb, x_d in ((xq_sb, xq_d), (xk_sb, xk_d), (xv_sb, xv_d)):
                for o in range(KSUB):
                    nc.sync.dma_start(
                        x_sb[:, o, :],
                        x_d.rearrange("(o p) s -> p o s", p=P)[:, o, :],
                    )

            # ---- persistent intermediates ----------------------------------
            qhT = proj.tile([P, NPAIR, SEQ], BF)  # [2-head dout, pair, seq]
            khT = proj.tile([P, NPAIR, SEQ], BF)
            vsb = proj.tile([P, NKT, HPC * DOUT], BF)  # v natural [seq, dout]
            ynT = proj.tile([P, NPAIR, SEQ], BF)  # normalized y.T

            vsb4 = vsb.rearrange("p k (h c) -> p k h c", c=DOUT)

            # ---- phase A helpers: projections ------------------------------
            # q/k head-pair-wise: psum[2*64 dout, 512 seq]
            # `which` selects q/k halves so the halves can be injected at
            # attention-block boundaries as dense PE re-warm work;
            # force_s uses the ps_s pool when the pv slots are held by a
            # live attention block's accumulators.
            def proj_qk(pair, which="qk", force_s=False):
                parts = {
                    "q": ((wq_sb, xq_sb, bqp_sb, qhT),),
                    "k": ((wk_sb, xk_sb, bkp_sb, khT),),
                    "qk": (
                        (wq_sb, xq_sb, bqp_sb, qhT),
                        (wk_sb, xk_sb, bkp_sb, khT),
                    ),
                }[which]
                for w_sb, x_sb, b_sb, o_sb in parts:
                    for qt in range(SEQ // 512):
                        if force_s:
                            pps = ps_s.tile(
                                [P, 1024], F32, tag="s", name="pps_s"
                            )[:, :512]
                        else:
                            pps = ps_pv.tile([P, 512], F32, tag="pv", name="pps")
                        for o in range(KSUB):
                            nc.tensor.matmul(
                                pps[:],
                                w_sb[:, o, pair * P : (pair + 1) * P],
                                x_sb[:, o, qt * 512 : (qt + 1) * 512],
                                start=(o == 0),
                                stop=(o == KSUB - 1),
                            )
                        nc.vector.tensor_tensor(
                            o_sb[:, pair, qt * 512 : (qt + 1) * 512],
                            pps[:],
                            b_sb[:, pair : pair + 1].to_broadcast((P, 512)),
                            ADD,
                        )

            # v natural layout: psum[128 seq, 256 dout]
            def proj_v():
                for st in range(NKT):
                    vps = ps_pv.tile([P, 512], F32, tag="pv", name="vps")
                    for o in range(KSUB):
                        nc.tensor.matmul(
                            vps[:, : HPC * DOUT],
                            xv_sb[:, o, st * P : (st + 1) * P],
                            wv_sb[:, o, :],
                            start=(o == 0),
                            stop=(o == KSUB - 1),
                        )
                    nc.vector.tensor_tensor(
                        vsb4[:, st, :, :],
                        vps[:, : HPC * DOUT].rearrange("p (h c) -> p h c", c=DOUT),
                        bvr_sb.rearrange("p (h c) -> p h c", c=DOUT),
                        ADD,
                    )

            # ---- phase B: attention ----------------------------------------
            # Deferred block epilogue, split into 3 stages that are spread
            # through the NEXT block's kt loop (so the PE never starves):
            #   stage 0: one [97,512] copy of the dn rows + one fast recip
            #   stage 1/2 (per qi): 2 col-packed K=1 broadcast matmuls of the
            #     reciprocal rows into rdb[128,512], then ONE full-height
            #     multiply pv_sb * rdb -> ynT (both head slices at once).
            pending = []

            def make_epilogue(dn_ps, pv_sbs, pair_, qh_):
                den = epi.tile([97, 512], F32, tag="den", name="den", bufs=2)
                rcp = epi.tile([97, 512], F32, tag="rcp", name="rcp", bufs=2)

                def stage0():
                    nc.vector.tensor_copy(den[:], dn_ps[0:97, :])
                    nc.vector.reciprocal_approx_fast(rcp[:], den[:])

                def stage_qi(qi):
                    def run():
                        q0 = (qh_ * 2 + qi) * 512
                        rdb = ps_s.tile([P, 512], F32, tag="s", name="rdb")
                        for h2 in range(2):
                            row = 32 * qi + 64 * h2
                            nc.tensor.matmul(
                                rdb[h2 * DOUT : (h2 + 1) * DOUT, :],
                                ones64_sb[row : row + 1, :],
                                rcp[row : row + 1, :],
                                start=True,
                                stop=True,
                                tile_position=(row, h2 * DOUT),
                                skip_group_check=True,
                            )
                        nc.vector.tensor_tensor(
                            ynT[:, pair_, q0 : q0 + 512],
                            pv_sbs[qi][:],
                            rdb[:],
                            MUL,
                        )

                    return run

                return [stage0, stage_qi(0), stage_qi(1)]

            def attn_block(pair, qh):
                if True:  # keep indentation
                    # accumulators are DVE-zeroed and every matmul uses
                    # start=False
                    pvs = []
                    for i in range(2):
                        pv = ps_pv.tile([P, 512], F32, tag="pv", name=f"pv{i}")
                        nc.vector.memset(pv[:], 0.0)
                        pvs.append(pv)
                    dn = ps_dn.tile([P, 512], F32, tag="dn", name="dn")
                    nc.vector.memset(dn[:], 0.0)

                    def pv_dn_mms(kt, ptms):
                        for qi in range(2):
                            for h2 in range(2):
                                # col-packed: head h2 -> psum partitions
                                # h2*64..h2*64+63
                                nc.tensor.matmul(
                                    pvs[qi][h2 * DOUT : (h2 + 1) * DOUT, :],
                                    vsb4[:, kt, pair * 2 + h2, :],
                                    ptms[h2][:, qi * 512 : (qi + 1) * 512],
                                    start=False,
                                    stop=(kt == NKT - 1),
                                    tile_position=(0, h2 * DOUT),
                                    skip_group_check=True,
                                )
                        for qi in range(2):
                            for h2 in range(2):
                                row = 32 * qi + 64 * h2
                                nc.tensor.matmul(
                                    dn[row : row + 1, :],
                                    ones_sb[:],
                                    ptms[h2][:, qi * 512 : (qi + 1) * 512],
                                    start=False,
                                    stop=(kt == NKT - 1),
                                    tile_position=(0, row),
                                    skip_group_check=True,
                                )

                    # software pipeline: PV/dn matmuls trail the S/softmax
                    # stage by LAG kts — the ACT+mask path takes ~2-3us from
                    # scores to masked probabilities, so a deep lag keeps the
                    # PE from starving (starved PE -> HAM half-clock).
                    LAG = 3
                    ptm_hist = []
                    for kt in range(NKT):
                        if pending and kt in (3, 5, 7, 9, 11, 12):
                            pending.pop(0)()
                        mt = maskp.tile([P, 1024], BF, tag="mt", name="mt")
                        nc.sync.dma_start(
                            mt[:],
                            mk_d[kt * P : (kt + 1) * P, qh * 1024 : (qh + 1) * 1024],
                        )
                        ptms = []
                        for h2 in range(2):
                            hs = slice(h2 * DOUT, (h2 + 1) * DOUT)
                            sps = ps_s.tile([P, 1024], F32, tag="s", name="sps")
                            for qi in range(2):
                                q0 = (qh * 2 + qi) * 512
                                nc.tensor.matmul(
                                    sps[:, qi * 512 : (qi + 1) * 512],
                                    khT[hs, pair, kt * P : (kt + 1) * P],
                                    qhT[hs, pair, q0 : q0 + 512],
                                    start=True,
                                    stop=True,
                                )
                            ptm = ptp.tile(
                                [P, 1024], BF, tag=f"ptm{h2}", name="ptm", bufs=4
                            )
                            if kt in STT_KTS:
                                # fused Schraudolph exp + additive mask on DVE
                                nc.vector.scalar_tensor_tensor(
                                    ptm.bitcast(I16)[:],
                                    sps[:],
                                    A_STT,
                                    mt[:],
                                    op0=MUL,
                                    op1=ADD,
                                )
                            else:
                                pt = ptp.tile(
                                    [P, 1024], BF, tag=f"pt{h2}", name="pt", bufs=3
                                )
                                nc.scalar.activation(
                                    pt[:], sps[:], EXP, scale=0.125, bias=actb_sb[:]
                                )
                                # 0/1 mask multiply: GpSimd takes most tiles
                                # (split in halves to cut per-op latency),
                                # DVE the rest (engine balance)
                                if (kt, h2) in GP_MASK:
                                    for q2 in range(2):
                                        cs = slice(q2 * 512, (q2 + 1) * 512)
                                        nc.gpsimd.tensor_tensor(
                                            ptm[:, cs], pt[:, cs], mt[:, cs], MUL
                                        )
                                else:
                                    nc.vector.tensor_tensor(
                                        ptm[:], pt[:], mt[:], MUL
                                    )
                            ptms.append(ptm)
                        ptm_hist.append(ptms)
                        if kt >= LAG:
                            pv_dn_mms(kt - LAG, ptm_hist[kt - LAG])
                    for kt in range(NKT - LAG, NKT):
                        pv_dn_mms(kt, ptm_hist[kt])
                    # drain PV accumulators to SBUF promptly to free their
                    # psum slots; normalization is deferred (see above).
                    pv_sbs = []
                    for qi in range(2):
                        pv_sb = epi.tile(
                            [P, 512], F32, tag="pvsb", name="pv_sb", bufs=4
                        )
                        # ScalarE is idle at block boundaries; draining there
                        # keeps the DVE free for the next block's memsets
                        nc.scalar.copy(pv_sb[:], pvs[qi][:])
                        pv_sbs.append(pv_sb)
                    pending.extend(make_epilogue(dn, pv_sbs, pair, qh))

            # ---- phase C: output projection (partial over 4 heads) ---------
            cidx = [0]

            def phase_c(st_range, force_s=False):
                for st in st_range:
                    for ntile in range(DIN // 512):
                        if cidx[0] % 2 == 0 and not force_s:
                            cps = ps_pv.tile([P, 512], F32, tag="pv", name="cps")
                        else:
                            cps = ps_s.tile(
                                [P, 1024], F32, tag="s", name="cps2"
                            )[:, :512]
                        for pair in range(NPAIR):
                            nc.tensor.matmul(
                                cps[:],
                                ynT[:, pair, st * P : (st + 1) * P],
                                wp_sb[:, pair, ntile * 512 : (ntile + 1) * 512],
                                start=(pair == 0),
                                stop=(pair == NPAIR - 1),
                            )
                        ost = epi.tile([P, 512], F32, tag="ost", name="ost", bufs=4)
                        if cidx[0] % 2 == 0:
                            nc.scalar.copy(ost[:], cps[:])
                        else:
                            nc.vector.tensor_copy(ost[:], cps[:])
                        nc.sync.dma_start(
                            out_d[
                                st * P : (st + 1) * P,
                                ntile * 512 : (ntile + 1) * 512,
                            ],
                            ost[:],
                        )
                        cidx[0] += 1

            # interleaved phase order. Every attention->attention boundary is
            # separated by a burst of dense dependency-light matmuls
            # (projection halves / output-projection groups / warm filler):
            # a PE stall at a boundary flips the HAM clock gate to 4/8 and
            # the whole next block then runs at half clock, so the bursts
            # both re-warm the gate and do useful work.
            proj_qk(0)
            proj_v()
            attn_block(0, 0)
            proj_qk(1, which="q")  # boundary burst (~4us dense)
            attn_block(0, 1)
            proj_qk(1, which="k")  # boundary burst
            attn_block(1, 0)
            # (1,0)|(1,1) boundary: the epilogue flush is DVE-heavy, so lace
            # warm filler matmuls through it, then run the first half of the
            # output projection (whose pair-1 operand the flush produces).
            warm_fill(6)
            pending.pop(0)()  # stage0: dn copy + reciprocal
            warm_fill(12)
            pending.pop(0)()  # stage_qi(0) -> ynT[:, 1, 0:512]
            phase_c(range(0, 4))
            pending.pop(0)()  # stage_qi(1) -> ynT[:, 1, 512:1024]
            phase_c(range(4, 8))
            attn_block(1, 1)
            while pending:
                pending.pop(0)()
            phase_c(range(8, NKT))

    nc.finalize()
    return nc


def make_in_maps(query, key, value, mask, Wq, bq, Wk, bk, Wv, bv, Wp, bp):
    """Shard + pre-layout the full inputs into 8 per-core input dicts."""
    in_maps = []
    # mask rows (seq_k blocks of 128) alternate between the multiplicative
    # 0/1 encoding (ACT-path kts) and the additive Schraudolph encoding
    # (STT-path kts); build once per batch element.
    mask_rows = {}
    for n in range(N):
        mT = np.ascontiguousarray(mask[n].T)  # [seq_k, seq_q]; True -> masked
        mx = np.empty((SEQ, SEQ), dtype=BF_NP)
        for kt in range(NKT):
            rows = slice(kt * P, (kt + 1) * P)
            if kt in STT_KTS:
                mx[rows] = np.where(mT[rows], B_MASKED, B_VALID).astype(BF_NP)
            else:
                mx[rows] = (~mT[rows]).astype(BF_NP)
        mask_rows[n] = mx

    for c in range(8):
        n = c // 4
        h0 = HPC * (c % 4)
        hs = slice(h0, h0 + HPC)

        def t_bf(x):  # [SEQ, DIN] -> contiguous [DIN, SEQ] bf16
            return np.ascontiguousarray(x.T).astype(BF_NP)

        # (H', DIN, DOUT) -> (DIN, H'*DOUT), head-major columns
        def w_bf(W):
            return np.ascontiguousarray(
                W[hs].transpose(1, 0, 2).reshape(DIN, HPC * DOUT)
            ).astype(BF_NP)

        # per-pair per-partition bias: [128, 2], col p = concat of heads (2p, 2p+1)
        def b_pair(b):
            return np.ascontiguousarray(b[hs].reshape(NPAIR, P).T).astype(np.float32)

        in_maps.append(
            {
                "xqT": t_bf(query[n]),
                "xkT": t_bf(key[n]),
                "xvT": t_bf(value[n]),
                "maskT": mask_rows[n],
                "wq": w_bf(Wq),
                "wk": w_bf(Wk),
                "wv": w_bf(Wv),
                "wp": np.ascontiguousarray(
                    Wp[h0 * DOUT : (h0 + HPC) * DOUT, :]
                ).astype(BF_NP),
                "bqp": b_pair(bq),
                "bkp": b_pair(bk),
                "bvr": np.ascontiguousarray(
                    np.tile(bv[hs].reshape(1, HPC * DOUT), (P, 1))
                ).astype(np.float32),
            }
        )
    return in_maps


def kernel(**inputs):
    global _NC_CACHE
    from concourse.bass_utils import run_bass_kernel_spmd

    if _NC_CACHE is None:
        _NC_CACHE = build_bass()
    nc = _NC_CACHE

    in_maps = make_in_maps(**inputs)
    res = run_bass_kernel_spmd(nc, in_maps, core_ids=list(range(8))).results
    parts = [res[c]["out"].astype(np.float32) for c in range(8)]
    bp = inputs["bp"]
    out = np.stack(
        [
            parts[0] + parts[1] + parts[2] + parts[3] + bp[None, :],
            parts[4] + parts[5] + parts[6] + parts[7] + bp[None, :],
        ]
    )
    return out.astype(np.float32)


# revision 22
# speedup vs baseline: 1.0684x; 1.0684x over previous
"""Multi-head attention (N=2, K=2048, DIN=1024, H=16, DOUT=64) on 8 TRN2 NeuronCores.

Sharding: data-parallel over batch N (cores 0-3 -> n=0, cores 4-7 -> n=1),
tensor-parallel over heads (4 heads per core). Each core computes its 4 heads'
attention plus the partial output projection over its head-feature slice of Wp;
the host sums the 4 partials per batch element and adds the output bias.

Per-core kernel structure (all matmuls bf16, fp32 PSUM accumulation):
  - host pre-transposes/casts activations to bf16 [DIN, SEQ] so projection
    matmuls contract over DIN on partitions with natural contiguous DMA.
  - q/k projected head-pair-wise to [dout(2 heads on partitions), seq];
    v projected to the natural [seq, 4*64] layout.
  - scores computed transposed (S.T = k_h @ q_h.T: seq_k on partitions), so
    softmax probabilities are already in the layout the PV matmul needs.
  - no max-subtraction: scores are bounded (~|S/8| < 10), exp is safe.
  - softmax numerator is produced by TWO engine paths, alternating per
    seq_k tile (kt) so ScalarE, VectorE and GpSimd all share the exp load
    instead of ScalarE serializing the whole attention phase:
      * ACT path (9 of 16 kts): ScalarE spline exp reads S from PSUM
        (scale=1/8 plus a mean-matching bias, see below); the 0/1 mask
        multiply runs on GpSimd for most tiles, VectorE for the rest.
      * STT path (7 of 16 kts): one fused VectorE scalar_tensor_tensor
        computes a Schraudolph fast-exp WITH the mask applied:
        int16(S*(2^7*log2e/8) + B[k,q]) bitcast as bf16, where the host
        baked B = 16256 (valid -> exp) / -57344 (masked -> the fp32->int16
        convert saturates to -32768 = 0x8000 = bf16 -0.0). The linear-
        mantissa approximation is exact in the exponent and overestimates
        by mean +4.07% in the mantissa; the ACT path's exp bias
        ln(1.0407) matches that mean so the two representations can mix
        within one softmax sum (verified: rel err 1.28e-2 vs 2e-2 budget).
    The host interleaves the two mask encodings row-block-wise in ONE
    [SEQ, SEQ] bf16 tensor, so each kt DMAs only the form it needs.
  - the two heads' PV matmuls col-pack into one PSUM bank; softmax
    denominators accumulate via col-packed M=1 ones-matmuls in a separate
    bank, doubling as PE filler that keeps the HAM clock gate at 8/8.
  - a dependency-free warmup burst of full-array matmuls at kernel start
    brings the clock gate to 8/8 before the first DMA-fed matmuls arrive.
  - PSUM accumulators are DVE-zeroed and all accumulating matmuls use
    start=False.
  - normalization: the 4 denominator rows (psum partitions 0/32/64/96) are
    pulled with ONE [97,512] copy + ONE fast-reciprocal, broadcast via K=1
    outer-product matmuls into a col-packed [128,512] tile, and applied
    with one full-height multiply per qi.
  - epilogues are deferred and spread through the NEXT block's matmul
    stream so the PE never idles long enough to re-throttle.
"""

import numpy as np
import ml_dtypes

import concourse.bass as bass
import concourse.mybir as mybir
from concourse import bacc
from concourse.tile import TileContext

P = 128
SEQ = 2048
DIN = 1024
DOUT = 64
H = 16
N = 2
HPC = 4  # heads per core
NPAIR = 2  # head pairs per core
KSUB = DIN // P  # 8 contraction subtiles for projections
NKT = SEQ // P  # 16 seq_k tiles of 128
BF = mybir.dt.bfloat16
F32 = mybir.dt.float32
I16 = mybir.dt.int16
BF_NP = ml_dtypes.bfloat16

# Schraudolph fast-exp constants (see module docstring)
A_STT = 0.125 * float(np.log2(np.e)) * 128.0  # 23.0831...
B_VALID = 16256.0  # 127 * 2^7, bf16-exact
B_MASKED = -57344.0  # saturates the int16 convert -> 0x8000 -> bf16 -0.0
ACT_BIAS = 0.039883  # ln(1.04068): mean ratio of fast-exp vs exp
# kts produced by the fused VectorE fast-exp path (rest go via ScalarE).
# kt 14 is STT and kt 15's masks run on DVE so the block tail's masked
# probabilities arrive with short latency (no PE stall at the boundary).
STT_KTS = frozenset((3, 5, 7, 9, 11, 14))
# ACT-path tiles whose 0/1 mask multiply runs on GpSimd (engine balance;
# kt 15 stays on DVE for tail latency)
GP_MASK = frozenset(
    [(kt, 0) for kt in (0, 2, 4, 6, 8, 10, 12, 13)] + [(0, 1), (2, 1)]
)

_NC_CACHE = None


def build_bass():
    nc = bacc.Bacc()

    xq_d = nc.declare_dram_parameter("xqT", [DIN, SEQ], BF, isOutput=False)
    xk_d = nc.declare_dram_parameter("xkT", [DIN, SEQ], BF, isOutput=False)
    xv_d = nc.declare_dram_parameter("xvT", [DIN, SEQ], BF, isOutput=False)
    mk_d = nc.declare_dram_parameter("maskT", [SEQ, SEQ], BF, isOutput=False)
    wq_d = nc.declare_dram_parameter("wq", [DIN, HPC * DOUT], BF, isOutput=False)
    wk_d = nc.declare_dram_parameter("wk", [DIN, HPC * DOUT], BF, isOutput=False)
    wv_d = nc.declare_dram_parameter("wv", [DIN, HPC * DOUT], BF, isOutput=False)
    wp_d = nc.declare_dram_parameter("wp", [HPC * DOUT, DIN], BF, isOutput=False)
    bqp_d = nc.declare_dram_parameter("bqp", [P, NPAIR], F32, isOutput=False)
    bkp_d = nc.declare_dram_parameter("bkp", [P, NPAIR], F32, isOutput=False)
    bvr_d = nc.declare_dram_parameter("bvr", [P, HPC * DOUT], F32, isOutput=False)
    out_d = nc.declare_dram_parameter("out", [SEQ, DIN], F32, isOutput=True)

    ADD = mybir.AluOpType.add
    MUL = mybir.AluOpType.mult
    EXP = mybir.ActivationFunctionType.Exp

    with TileContext(nc) as tc:
        with (
            tc.tile_pool(name="const", bufs=1) as const,
            tc.tile_pool(name="xin", bufs=1) as xin,
            tc.tile_pool(name="proj", bufs=1) as proj,
            tc.tile_pool(name="maskp", bufs=5) as maskp,
            tc.tile_pool(name="ptp", bufs=2) as ptp,
            tc.tile_pool(name="epi", bufs=2) as epi,
            tc.tile_pool(name="ps_s", bufs=2, space="PSUM") as ps_s,
            tc.tile_pool(name="ps_pv", bufs=2, space="PSUM") as ps_pv,
            tc.tile_pool(name="ps_dn", bufs=1, space="PSUM") as ps_dn,
            tc.tile_pool(name="ps_tr", bufs=1, space="PSUM") as ps_tr,
        ):
            # ---- constants -------------------------------------------------
            wq_sb = const.tile([P, KSUB, HPC * DOUT], BF)
            nc.sync.dma_start(wq_sb[:], wq_d.rearrange("(o p) m -> p o m", p=P))
            wk_sb = const.tile([P, KSUB, HPC * DOUT], BF)
            nc.sync.dma_start(wk_sb[:], wk_d.rearrange("(o p) m -> p o m", p=P))
            wv_sb = const.tile([P, KSUB, HPC * DOUT], BF)
            nc.sync.dma_start(wv_sb[:], wv_d.rearrange("(o p) m -> p o m", p=P))
            wp_sb = const.tile([P, NPAIR, DIN], BF)
            nc.sync.dma_start(wp_sb[:], wp_d.rearrange("(o p) n -> p o n", p=P))
            bqp_sb = const.tile([P, NPAIR], F32)
            nc.sync.dma_start(bqp_sb[:], bqp_d[:])
            bkp_sb = const.tile([P, NPAIR], F32)
            nc.sync.dma_start(bkp_sb[:], bkp_d[:])
            bvr_sb = const.tile([P, HPC * DOUT], F32)
            nc.sync.dma_start(bvr_sb[:], bvr_d[:])
            ones_sb = const.tile([P, 1], BF)
            nc.vector.memset(ones_sb[:], 1.0)
            # per-partition bias AP for the ACT-path exp (mean-matching)
            actb_sb = const.tile([P, 1], F32)
            nc.vector.memset(actb_sb[:], ACT_BIAS)
            # full-height fp32 ones: rows at 0/32/64/96 serve as the K=1
            # outer-product lhsT for the denominator broadcast
            ones64_sb = const.tile([P, DOUT], F32)
            nc.vector.memset(ones64_sb[:], 1.0)

            # Dependency-free filler matmuls into a dedicated never-read PSUM
            # bank. Used (a) as a warmup burst at kernel start so the HAM
            # clock gate reaches 8/8 before the first DMA-fed matmuls, and
            # (b) laced through the attention blocks: whenever the real
            # stream hiccups for ~0.5us on a softmax dependency, the PE
            # chews filler instead of idling -- an idle gap re-throttles the
            # clock gate and taxes the next several us at half clock.
            warm_rhs = const.tile([P, 512], BF)
            nc.vector.memset(warm_rhs[:], 1.0)
            trash_ps = ps_tr.tile([P, 512], F32, tag="tr", name="trash")

            def warm_fill(n, nn=256):
                for i in range(n):
                    nc.tensor.matmul(
                        trash_ps[:, 0:nn],
                        warm_rhs[:, 0:P],
                        warm_rhs[:, 0:nn],
                        start=True,
                        stop=True,
                        skip_group_check=True,
                    )

            warm_fill(80)

            # ---- resident transposed activations ---------------------------
            # chunked per DIN-subtile so the first projection matmuls can
            # start as soon as the first 512KB lands
            xq_sb = xin.tile([P, KSUB, SEQ], BF)
            xk_sb = xin.tile([P, KSUB, SEQ], BF)
            xv_sb = xin.tile([P, KSUB, SEQ], BF)
            for x_sb, x_d in ((xq_sb, xq_d), (xk_sb, xk_d), (xv_sb, xv_d)):
                for o in range(KSUB):
                    nc.sync.dma_start(
                        x_sb[:, o, :],
                        x_d.rearrange("(o p) s -> p o s", p=P)[:, o, :],
                    )

            # ---- persistent intermediates ----------------------------------
            qhT = proj.tile([P, NPAIR, SEQ], BF)  # [2-head dout, pair, seq]
            khT = proj.tile([P, NPAIR, SEQ], BF)
            vsb = proj.tile([P, NKT, HPC * DOUT], BF)  # v natural [seq, dout]
            ynT = proj.tile([P, NPAIR, SEQ], BF)  # normalized y.T

            vsb4 = vsb.rearrange("p k (h c) -> p k h c", c=DOUT)

            # ---- phase A helpers: projections ------------------------------
            # q/k head-pair-wise: psum[2*64 dout, 512 seq]
            # `which` selects q/k halves so the halves can be injected at
            # attention-block boundaries as dense PE re-warm work;
            # force_s uses the ps_s pool when the pv slots are held by a
            # live attention block's accumulators.
            def proj_qk(pair, which="qk", force_s=False):
                parts = {
                    "q": ((wq_sb, xq_sb, bqp_sb, qhT),),
                    "k": ((wk_sb, xk_sb, bkp_sb, khT),),
                    "qk": (
                        (wq_sb, xq_sb, bqp_sb, qhT),
                        (wk_sb, xk_sb, bkp_sb, khT),
                    ),
                }[which]
                for w_sb, x_sb, b_sb, o_sb in parts:
                    for qt in range(SEQ // 512):
                        if force_s:
                            pps = ps_s.tile(
                                [P, 1024], F32, tag="s", name="pps_s"
                            )[:, :512]
                        else:
                            pps = ps_pv.tile([P, 512], F32, tag="pv", name="pps")
                        for o in range(KSUB):
                            nc.tensor.matmul(
                                pps[:],
                                w_sb[:, o, pair * P : (pair + 1) * P],
                                x_sb[:, o, qt * 512 : (qt + 1) * 512],
                                start=(o == 0),
                                stop=(o == KSUB - 1),
                            )
                        nc.vector.tensor_tensor(
                            o_sb[:, pair, qt * 512 : (qt + 1) * 512],
                            pps[:],
                            b_sb[:, pair : pair + 1].to_broadcast((P, 512)),
                            ADD,
                        )

            # v natural layout: psum[128 seq, 256 dout]
            def proj_v():
                for st in range(NKT):
                    vps = ps_pv.tile([P, 512], F32, tag="pv", name="vps")
                    for o in range(KSUB):
                        nc.tensor.matmul(
                            vps[:, : HPC * DOUT],
                            xv_sb[:, o, st * P : (st + 1) * P],
                            wv_sb[:, o, :],
                            start=(o == 0),
                            stop=(o == KSUB - 1),
                        )
                    nc.vector.tensor_tensor(
                        vsb4[:, st, :, :],
                        vps[:, : HPC * DOUT].rearrange("p (h c) -> p h c", c=DOUT),
                        bvr_sb.rearrange("p (h c) -> p h c", c=DOUT),
                        ADD,
                    )

            # ---- phase B: attention ----------------------------------------
            # Deferred block epilogue, split into 3 stages that are spread
            # through the NEXT block's kt loop (so the PE never starves):
            #   stage 0: one [97,512] copy of the dn rows + one fast recip
            #   stage 1/2 (per qi): 2 col-packed K=1 broadcast matmuls of the
            #     reciprocal rows into rdb[128,512], then ONE full-height
            #     multiply pv_sb * rdb -> ynT (both head slices at once).
            pending = []

            def make_epilogue(dn_ps, pv_sbs, pair_, qh_):
                den = epi.tile([97, 512], F32, tag="den", name="den", bufs=2)
                rcp = epi.tile([97, 512], F32, tag="rcp", name="rcp", bufs=2)

                def stage0():
                    nc.vector.tensor_copy(den[:], dn_ps[0:97, :])
                    nc.vector.reciprocal_approx_fast(rcp[:], den[:])

                def stage_qi(qi):
                    def run():
                        q0 = (qh_ * 2 + qi) * 512
                        rdb = ps_s.tile([P, 512], F32, tag="s", name="rdb")
                        for h2 in range(2):
                            row = 32 * qi + 64 * h2
                            nc.tensor.matmul(
                                rdb[h2 * DOUT : (h2 + 1) * DOUT, :],
                                ones64_sb[row : row + 1, :],
                                rcp[row : row + 1, :],
                                start=True,
                                stop=True,
                                tile_position=(row, h2 * DOUT),
                                skip_group_check=True,
                            )
                        nc.vector.tensor_tensor(
                            ynT[:, pair_, q0 : q0 + 512],
                            pv_sbs[qi][:],
                            rdb[:],
                            MUL,
                        )

                    return run

                # stage0 runs at the block boundary (the dn bank is single-
                # buffered: the next block's memset must wait for it anyway);
                # only the normalize stages are deferred into the next block.
                stage0()
                return [stage_qi(0), stage_qi(1)]

            def attn_block(pair, qh):
                if True:  # keep indentation
                    # accumulators are DVE-zeroed and every matmul uses
                    # start=False
                    pvs = []
                    for i in range(2):
                        pv = ps_pv.tile([P, 512], F32, tag="pv", name=f"pv{i}")
                        nc.vector.memset(pv[:], 0.0)
                        pvs.append(pv)
                    dn = ps_dn.tile([P, 512], F32, tag="dn", name="dn")
                    nc.vector.memset(dn[:], 0.0)

                    def pv_dn_mms(kt, ptms):
                        for qi in range(2):
                            for h2 in range(2):
                                # col-packed: head h2 -> psum partitions
                                # h2*64..h2*64+63
                                nc.tensor.matmul(
                                    pvs[qi][h2 * DOUT : (h2 + 1) * DOUT, :],
                                    vsb4[:, kt, pair * 2 + h2, :],
                                    ptms[h2][:, qi * 512 : (qi + 1) * 512],
                                    start=False,
                                    stop=(kt == NKT - 1),
                                    tile_position=(0, h2 * DOUT),
                                    skip_group_check=True,
                                )
                        for qi in range(2):
                            for h2 in range(2):
                                row = 32 * qi + 64 * h2
                                nc.tensor.matmul(
                                    dn[row : row + 1, :],
                                    ones_sb[:],
                                    ptms[h2][:, qi * 512 : (qi + 1) * 512],
                                    start=False,
                                    stop=(kt == NKT - 1),
                                    tile_position=(0, row),
                                    skip_group_check=True,
                                )

                    # software pipeline: PV/dn matmuls trail the S/softmax
                    # stage by LAG kts — the ACT+mask path takes ~2-3us from
                    # scores to masked probabilities, so a deep lag keeps the
                    # PE from starving (starved PE -> HAM half-clock).
                    LAG = 3
                    PREF = 2  # mask DMA prefetch depth (kts)

                    def mt_dma(kt):
                        mt = maskp.tile([P, 1024], BF, tag="mt", name="mt")
                        nc.sync.dma_start(
                            mt[:],
                            mk_d[kt * P : (kt + 1) * P, qh * 1024 : (qh + 1) * 1024],
                        )
                        return mt

                    mts = {kt: mt_dma(kt) for kt in range(PREF)}
                    ptm_hist = []
                    dve_masks = []  # deferred DVE mask multiplies
                    for kt in range(NKT):
                        if kt + PREF < NKT:
                            mts[kt + PREF] = mt_dma(kt + PREF)
                        # dependency-free filler so a softmax hiccup doesn't
                        # leave the PE idle (idle -> clock-gate re-throttle)
                        if kt >= 1:
                            warm_fill(1, nn=512)
                        mt = mts.pop(kt)
                        ptms = []
                        for h2 in range(2):
                            hs = slice(h2 * DOUT, (h2 + 1) * DOUT)
                            sps = ps_s.tile([P, 1024], F32, tag="s", name="sps")
                            for qi in range(2):
                                q0 = (qh * 2 + qi) * 512
                                nc.tensor.matmul(
                                    sps[:, qi * 512 : (qi + 1) * 512],
                                    khT[hs, pair, kt * P : (kt + 1) * P],
                                    qhT[hs, pair, q0 : q0 + 512],
                                    start=True,
                                    stop=True,
                                )
                            ptm = ptp.tile(
                                [P, 1024], BF, tag=f"ptm{h2}", name="ptm", bufs=4
                            )
                            if kt in STT_KTS:
                                # fused Schraudolph exp + additive mask on DVE
                                nc.vector.scalar_tensor_tensor(
                                    ptm.bitcast(I16)[:],
                                    sps[:],
                                    A_STT,
                                    mt[:],
                                    op0=MUL,
                                    op1=ADD,
                                )
                            else:
                                pt = ptp.tile(
                                    [P, 1024], BF, tag=f"pt{h2}", name="pt", bufs=3
                                )
                                nc.scalar.activation(
                                    pt[:], sps[:], EXP, scale=0.125, bias=actb_sb[:]
                                )
                                # 0/1 mask multiply: GpSimd takes most tiles
                                # (split in halves to cut per-op latency).
                                # DVE's share is DEFERRED one kt: issued now
                                # it would head-of-line-block the DVE queue
                                # behind ScalarE's exp, delaying the next
                                # kt's sps-freeing fast-exp.
                                if (kt, h2) in GP_MASK:
                                    for q2 in range(2):
                                        cs = slice(q2 * 512, (q2 + 1) * 512)
                                        nc.gpsimd.tensor_tensor(
                                            ptm[:, cs], pt[:, cs], mt[:, cs], MUL
                                        )
                                else:
                                    dve_masks.append((ptm, pt, mt))
                            ptms.append(ptm)
                        ptm_hist.append(ptms)
                        while dve_masks and (kt in STT_KTS or kt == NKT - 1):
                            m_out, m_in, m_mt = dve_masks.pop(0)
                            nc.vector.tensor_tensor(m_out[:], m_in[:], m_mt[:], MUL)
                        if kt >= LAG:
                            pv_dn_mms(kt - LAG, ptm_hist[kt - LAG])
                        if pending and kt in (3, 5, 7, 9, 11, 12):
                            pending.pop(0)()
                    for kt in range(NKT - LAG, NKT):
                        pv_dn_mms(kt, ptm_hist[kt])
                    # drain PV accumulators to SBUF promptly to free their
                    # psum slots; normalization is deferred (see above).
                    pv_sbs = []
                    for qi in range(2):
                        pv_sb = epi.tile(
                            [P, 512], F32, tag="pvsb", name="pv_sb", bufs=3
                        )
                        # ScalarE is idle at block boundaries; draining there
                        # keeps the DVE free for the next block's memsets
                        nc.scalar.copy(pv_sb[:], pvs[qi][:])
                        pv_sbs.append(pv_sb)
                    pending.extend(make_epilogue(dn, pv_sbs, pair, qh))

            # ---- phase C: output projection (partial over 4 heads) ---------
            cidx = [0]

            def phase_c(st_range, force_s=False):
                for st in st_range:
                    for ntile in range(DIN // 512):
                        if cidx[0] % 2 == 0 and not force_s:
                            cps = ps_pv.tile([P, 512], F32, tag="pv", name="cps")
                        else:
                            cps = ps_s.tile(
                                [P, 1024], F32, tag="s", name="cps2"
                            )[:, :512]
                        for pair in range(NPAIR):
                            nc.tensor.matmul(
                                cps[:],
                                ynT[:, pair, st * P : (st + 1) * P],
                                wp_sb[:, pair, ntile * 512 : (ntile + 1) * 512],
                                start=(pair == 0),
                                stop=(pair == NPAIR - 1),
                            )
                        ost = epi.tile([P, 512], F32, tag="ost", name="ost", bufs=3)
                        if cidx[0] % 2 == 0:
                            nc.scalar.copy(ost[:], cps[:])
                        else:
                            nc.vector.tensor_copy(ost[:], cps[:])
                        nc.sync.dma_start(
                            out_d[
                                st * P : (st + 1) * P,
                                ntile * 512 : (ntile + 1) * 512,
                            ],
                            ost[:],
                        )
                        cidx[0] += 1

            # interleaved phase order. Every attention->attention boundary is
            # separated by a burst of dense dependency-light matmuls
            # (projection halves / output-projection groups / warm filler):
            # a PE stall at a boundary flips the HAM clock gate to 4/8 and
            # the whole next block then runs at half clock, so the bursts
            # both re-warm the gate and do useful work.
            proj_qk(0)
            proj_v()
            attn_block(0, 0)
            proj_qk(1, which="q")  # boundary burst (~4us dense)
            attn_block(0, 1)
            proj_qk(1, which="k")  # boundary burst
            attn_block(1, 0)
            # (1,0)|(1,1) boundary: the epilogue flush is DVE-heavy, so lace
            # warm filler matmuls through it, then run the first half of the
            # output projection (whose pair-1 operand the flush produces).
            warm_fill(10)
            pending.pop(0)()  # stage_qi(0) -> ynT[:, 1, 0:512]
            phase_c(range(0, 4))
            pending.pop(0)()  # stage_qi(1) -> ynT[:, 1, 512:1024]
            phase_c(range(4, 8))
            attn_block(1, 1)
            while pending:
                pending.pop(0)()
            phase_c(range(8, NKT))

    nc.finalize()
    return nc


def make_in_maps(query, key, value, mask, Wq, bq, Wk, bk, Wv, bv, Wp, bp):
    """Shard + pre-layout the full inputs into 8 per-core input dicts."""
    in_maps = []
    # mask rows (seq_k blocks of 128) alternate between the multiplicative
    # 0/1 encoding (ACT-path kts) and the additive Schraudolph encoding
    # (STT-path kts); build once per batch element.
    mask_rows = {}
    for n in range(N):
        mT = np.ascontiguousarray(mask[n].T)  # [seq_k, seq_q]; True -> masked
        mx = np.empty((SEQ, SEQ), dtype=BF_NP)
        for kt in range(NKT):
            rows = slice(kt * P, (kt + 1) * P)
            if kt in STT_KTS:
                mx[rows] = np.where(mT[rows], B_MASKED, B_VALID).astype(BF_NP)
            else:
                mx[rows] = (~mT[rows]).astype(BF_NP)
        mask_rows[n] = mx

    for c in range(8):
        n = c // 4
        h0 = HPC * (c % 4)
        hs = slice(h0, h0 + HPC)

        def t_bf(x):  # [SEQ, DIN] -> contiguous [DIN, SEQ] bf16
            return np.ascontiguousarray(x.T).astype(BF_NP)

        # (H', DIN, DOUT) -> (DIN, H'*DOUT), head-major columns
        def w_bf(W):
            return np.ascontiguousarray(
                W[hs].transpose(1, 0, 2).reshape(DIN, HPC * DOUT)
            ).astype(BF_NP)

        # per-pair per-partition bias: [128, 2], col p = concat of heads (2p, 2p+1)
        def b_pair(b):
            return np.ascontiguousarray(b[hs].reshape(NPAIR, P).T).astype(np.float32)

        in_maps.append(
            {
                "xqT": t_bf(query[n]),
                "xkT": t_bf(key[n]),
                "xvT": t_bf(value[n]),
                "maskT": mask_rows[n],
                "wq": w_bf(Wq),
                "wk": w_bf(Wk),
                "wv": w_bf(Wv),
                "wp": np.ascontiguousarray(
                    Wp[h0 * DOUT : (h0 + HPC) * DOUT, :]
                ).astype(BF_NP),
                "bqp": b_pair(bq),
                "bkp": b_pair(bk),
                "bvr": np.ascontiguousarray(
                    np.tile(bv[hs].reshape(1, HPC * DOUT), (P, 1))
                ).astype(np.float32),
            }
        )
    return in_maps


def kernel(**inputs):
    global _NC_CACHE
    from concourse.bass_utils import run_bass_kernel_spmd

    if _NC_CACHE is None:
        _NC_CACHE = build_bass()
    nc = _NC_CACHE

    in_maps = make_in_maps(**inputs)
    res = run_bass_kernel_spmd(nc, in_maps, core_ids=list(range(8))).results
    parts = [res[c]["out"].astype(np.float32) for c in range(8)]
    bp = inputs["bp"]
    out = np.stack(
        [
            parts[0] + parts[1] + parts[2] + parts[3] + bp[None, :],
            parts[4] + parts[5] + parts[6] + parts[7] + bp[None, :],
        ]
    )
    return out.astype(np.float32)
